# revision 19
# baseline (speedup 1.0000x reference)
"""AtnConv (contextual attention) on 8 Trainium2 NeuronCores via Bass/Tile.

Sharding: data-parallel over B=4 samples x 2-way spatial split of the
64x64 score grid (rows). The softmax is over the patch axis L, which is
kept whole per core, so no collectives are needed. The row-half cores
(half=1) receive vertically flipped inputs so that a single SPMD program
with identical compile-time geometry serves both halves; the host flips
their outputs back.

The big operands are derived on device from the raw (padded, bf16)
feature maps to keep the host->device transfer small:
  - x2 im2col tiles PT[k,l] via 9 shifted DMAs (k ordered tap-major),
  - normalized keys wn = PT * nm (nm = mm/max(norm,1e-4), host-sent
    pre-broadcast), and
  - value patches A'[l,c] = x1 taps, transposed on the tensor engine
    with the mask/4 fold applied per-partition at PSUM evacuation.

Per core pipeline (sp_shard = 38 rows x 64 = 2432 positions, 19 blocks):
  phase 1: scores z[sp,l] = PT_sp.T @ wn (bf16 GEMM, fp32 accum),
           z *= 10*ma at evac, softmax along free dim l, scale by
           ma/denom, transpose to [l,sp] via tensor engine, spill to DRAM.
  phase 2: value GEMM P[c,sp] = A'.T @ yi over all L=4096 patches
           (16 (dh,dw) taps in 2 halves), scatter-add into y.
  phase 3: 4 dilated 3x3 conv branches + bias + ReLU as GEMMs.
"""

import sys
import numpy as np

sys.path.insert(0, "/opt/trn_rl_repo")
import ml_dtypes

BF16 = ml_dtypes.bfloat16

B, C1, H1, W1 = 4, 128, 128, 128
C2, H2, W2 = 64, 64, 64
SCALE = 10.0
GROUPS, OUT_C = 4, 64
RATES = (1, 2, 4, 8)
L = H2 * W2                      # 4096
NH = 38                          # h rows per shard (incl. halo)
NB = NH // 2                     # 19 blocks of 128 sp positions
SPS = NB * 128                   # 2432
KP = 5                           # 576 -> 5 k-tiles of 128
YROWS, YCOLS = 96, 148           # y buffer geometry (row idx = r+9, col idx = s+9)

_PROG = None
DEBUG = False


def _build_program(phases=3):
    import concourse.bacc as bacc
    import concourse.tile as tile
    import concourse.mybir as mybir
    from contextlib import ExitStack

    f32 = mybir.dt.float32
    bf = mybir.dt.bfloat16
    nc = bacc.Bacc("TRN2", target_bir_lowering=False, debug=False, num_devices=8)

    x1T_d = nc.dram_tensor("x1T", [130, 130, 128], bf, kind="ExternalInput").ap()
    x2p_d = nc.dram_tensor("x2p", [64, 66, 66], bf, kind="ExternalInput").ap()
    nmr_d = nc.dram_tensor("nmr", [1, L], bf, kind="ExternalInput").ap()
    mmr_d = nc.dram_tensor("mmr", [1, L], bf, kind="ExternalInput").ap()
    ma10_d = nc.dram_tensor("ma10", [128, NB], f32, kind="ExternalInput").ap()
    maS_d = nc.dram_tensor("maS", [128, NB], f32, kind="ExternalInput").ap()
    cw_d = nc.dram_tensor("convw", [128, 36, 16], bf, kind="ExternalInput").ap()
    cb_d = nc.dram_tensor("convb", [16, 4], f32, kind="ExternalInput").ap()
    out_d = nc.dram_tensor("out", [4, 16, 8192], f32, kind="ExternalOutput").ap()
    if DEBUG:
        ydbg_d = nc.dram_tensor("ydbg", [128, YROWS, YCOLS], f32, kind="ExternalOutput").ap()

    Exp = mybir.ActivationFunctionType.Exp
    Copy = mybir.ActivationFunctionType.Copy
    Relu = mybir.ActivationFunctionType.Relu
    Ax = mybir.AxisListType.X
    mult = mybir.AluOpType.mult
    add = mybir.AluOpType.add

    with tile.TileContext(nc) as tc, ExitStack() as es:
        dramp = es.enter_context(tc.tile_pool(name="dram", bufs=1, space="DRAM"))
        yib_t = dramp.tile([NB, 128, L], bf)
        cstp = es.enter_context(tc.tile_pool(name="cst", bufs=1))
        cw_sb = cstp.tile([128, 36, 16], bf)
        cb_sb = cstp.tile([16, 4], f32)
        nc.sync.dma_start(cw_sb[:], cw_d[:])
        nc.sync.dma_start(cb_sb[:], cb_d[:])
        # ---------------- phase 1: scores + softmax + transpose ----------
        with tc.tile_pool(name="c1", bufs=1) as c1:
            pt_sb = [c1.tile([128, SPS], bf, name=f"pt{k}") for k in range(KP)]
            wn_sb = [c1.tile([128, L], bf, name=f"wn{k}") for k in range(KP)]
            ma10_sb = c1.tile([128, NB], f32)
            ma_sb = c1.tile([128, NB], f32)
            mmb_sb = c1.tile([128, L], bf)
            mmr_sb = c1.tile([1, L], bf)
            nc.sync.dma_start(ma10_sb[:], ma10_d[:])
            nc.sync.dma_start(ma_sb[:], maS_d[:])
            nc.sync.dma_start(mmr_sb[:], mmr_d[:])
            nc.gpsimd.partition_broadcast(mmb_sb[:], mmr_sb[:])
            # im2col of x2 (k tap-major) via shifted DMAs: full columns into
            # wn tiles (then *= nm in place), first SPS columns again as lhsT
            nc.vector.memset(pt_sb[4][64:128, :], 0.0)
            nc.vector.memset(wn_sb[4][64:128, :], 0.0)
            for t in range(9):
                da, db = t // 3, t % 3
                u = (t % 2) * 64
                dstw = wn_sb[t // 2][u:u + 64, :].rearrange("p (h w) -> p h w", h=64)
                nc.sync.dma_start(dstw, x2p_d[:, da:da + 64, db:db + 64])
                dstp = pt_sb[t // 2][u:u + 64, :].rearrange("p (h w) -> p h w", h=NH)
                nc.sync.dma_start(dstp, x2p_d[:, da:da + NH, db:db + 64])
            with tc.tile_pool(name="nmp", bufs=1) as nmp:
                nmb_sb = nmp.tile([128, L], bf)
                nmr_sb = nmp.tile([1, L], bf)
                nc.sync.dma_start(nmr_sb[:], nmr_d[:])
                nc.gpsimd.partition_broadcast(nmb_sb[:], nmr_sb[:])
                for k in range(KP):
                    nc.vector.tensor_tensor(wn_sb[k][:], wn_sb[k][:], nmb_sb[:], op=mult)

            with (
                tc.tile_pool(name="sps", bufs=4, space="PSUM") as sps,
                tc.tile_pool(name="zp", bufs=2) as zp,
                tc.tile_pool(name="yib", bufs=2) as yibp,
                tc.tile_pool(name="st", bufs=12) as st,
            ):
                for b in range(NB):
                    z_sb = zp.tile([128, L], f32)
                    for nt in range(8):
                        ps = sps.tile([128, 512], f32)
                        for k in range(KP):
                            nc.tensor.matmul(
                                ps[:],
                                pt_sb[k][:, b * 128:(b + 1) * 128],
                                wn_sb[k][:, nt * 512:(nt + 1) * 512],
                                start=(k == 0),
                                stop=(k == KP - 1),
                            )
                        nc.scalar.activation(
                            z_sb[:, nt * 512:(nt + 1) * 512], ps[:], Copy,
                            scale=ma10_sb[:, b:b + 1],
                        )
                    nmax = st.tile([128, 1], f32)
                    nc.vector.reduce_max(nmax[:], z_sb[:], axis=Ax, negate=True)
                    yib = yibp.tile([128, L], bf)
                    den = st.tile([128, 1], f32)
                    nc.scalar.activation(
                        yib[:], z_sb[:], Exp, bias=nmax[:], accum_out=den[:]
                    )
                    rec = st.tile([128, 1], f32)
                    nc.vector.reciprocal(rec[:], den[:])
                    sc2 = st.tile([128, 1], f32)
                    nc.vector.tensor_tensor(
                        sc2[:], rec[:], ma_sb[:, b:b + 1], op=mult
                    )
                    nc.vector.scalar_tensor_tensor(
                        yib[:], yib[:], sc2[:], mmb_sb[:], op0=mult, op1=mult
                    )
                    nc.sync.dma_start(yib_t[b], yib[:])

        # ---------------- phase 2: value GEMM + scatter-add --------------
        yp = es.enter_context(tc.tile_pool(name="ybuf", bufs=1))
        y_sb = yp.tile([128, YROWS, YCOLS], f32)
        nc.vector.memset(y_sb[:], 0.0)
        if phases >= 2:
         with tc.tile_pool(name="ap", bufs=1) as apl:
            with (
                tc.tile_pool(name="yt4", bufs=2) as yt4p,
                tc.tile_pool(name="vps", bufs=4, space="PSUM") as vps,
            ):
                for H in range(2):
                    a_sb = apl.tile([128, 8, 32, 128], bf, tag="a")
                    # build A' half H via 16 transposing DMAs from x1T
                    for d8 in range(8):
                        d = H * 8 + d8
                        dh, dw = d // 4, d % 4
                        for hh in range(2):
                            src = x1T_d[dh + 2 * hh:dh + 2 * hh + 125:4,
                                        dw:dw + 127:2, :]
                            nc.sync.dma_start(
                                a_sb[hh * 64:(hh + 1) * 64, d8, :, :],
                                src.rearrange("lt w c -> w lt c"),
                            )
                    b0 = 0
                    for sb_i in range(5):
                        nb_sb = 4 if sb_i < 4 else 3
                        nsp = nb_sb * 128
                        yt4 = yt4p.tile([128, 32, 512], bf)
                        for lt in range(32):
                            src = yib_t[b0:b0 + nb_sb, :, lt * 128:(lt + 1) * 128]
                            nc.sync.dma_start(
                                yt4[:, lt, :nsp],
                                src.rearrange("j s p -> p (j s)"),
                            )
                        for d8 in range(8):
                            d = H * 8 + d8
                            dh, dw = d // 4, d % 4
                            pv = vps.tile([128, 512], f32)
                            for lt in range(32):
                                nc.tensor.matmul(
                                    pv[:, :nsp],
                                    a_sb[:, d8, lt, :],
                                    yt4[:, lt, :nsp],
                                    start=(lt == 0),
                                    stop=(lt == 31),
                                )
                            pvv = pv.rearrange("p (j w) -> p j w", w=128)
                            for hl in range(2):
                                r0 = 4 * b0 + 2 * hl + dh + 8
                                nc.vector.tensor_tensor(
                                    y_sb[:, r0:r0 + 4 * nb_sb:4, 8 + dw:8 + dw + 128:2],
                                    y_sb[:, r0:r0 + 4 * nb_sb:4, 8 + dw:8 + dw + 128:2],
                                    pvv[:, :nb_sb, hl * 64:hl * 64 + 64],
                                    op=add,
                                )
                        b0 += nb_sb
            # zero the out-of-image scatter junk (r=-1, s=-1, s=128 bands)
            nc.vector.memset(y_sb[:, 8, :], 0.0)
            nc.vector.memset(y_sb[:, :, 8], 0.0)
            nc.vector.memset(y_sb[:, :, 137], 0.0)
        if DEBUG:
            nc.sync.dma_start(ydbg_d[:], y_sb[:])

        # ---------------- phase 3: dilated conv branches -----------------
        if phases < 3:
            with tc.tile_pool(name="og0", bufs=1) as og0p:
                og0 = og0p.tile([16, 8192], f32)
                nc.vector.memset(og0[:], 0.0)
                for g in range(GROUPS):
                    nc.sync.dma_start(out_d[g], og0[:])
        else:
         with (
            tc.tile_pool(name="ybf", bufs=1) as ybfp,
            tc.tile_pool(name="osb", bufs=2) as osbp,
            tc.tile_pool(name="cps", bufs=4, space="PSUM") as cps,
        ):
            ybf = ybfp.tile([128, YROWS, YCOLS], bf)
            nc.vector.tensor_copy(ybf[:], y_sb[:])
            for g in range(GROUPS):
                rg = RATES[g]
                og = osbp.tile([16, 8192], f32, tag="og")
                for rb in range(16):
                    pc = cps.tile([16, 512], f32)
                    for a in range(3):
                        for b2 in range(3):
                            t = g * 9 + a * 3 + b2
                            r0 = rb * 4 + 9 + (a - 1) * rg
                            cs = 9 + (b2 - 1) * rg
                            nc.tensor.matmul(
                                pc[:],
                                cw_sb[:, t, :],
                                ybf[:, r0:r0 + 4, cs:cs + 128],
                                start=(a == 0 and b2 == 0),
                                stop=(a == 2 and b2 == 2),
                            )
                    nc.scalar.activation(
                        og[:, rb * 512:(rb + 1) * 512], pc[:], Relu,
                        bias=cb_sb[:, g:g + 1],
                    )
                nc.sync.dma_start(out_d[g], og[:])

    nc.compile()
    return nc


def _prep_core(x1s, x2s, masks, mas, conv_w, conv_b, flip):
    """Build the per-core input map. Inputs are per-sample [C,H,W] fp32."""
    if flip:
        x1s = x1s[:, ::-1, :]
        x2s = x2s[:, ::-1, :]
        masks = masks[:, ::-1, :]
        mas = mas[::-1, :]
        conv_w = conv_w[:, :, :, ::-1, :]

    x1T = np.zeros((130, 130, C1), np.float32)
    x1T[1:-1, 1:-1, :] = x1s.transpose(1, 2, 0)
    x2p = np.zeros((C2, 66, 66), np.float32)
    x2p[:, 1:-1, 1:-1] = x2s

    # mm from mask patches (4x4 stride 2 mean == 0)
    mp = np.zeros((1, H1 + 2, W1 + 2), np.float32)
    mp[:, 1:-1, 1:-1] = masks
    mv = np.lib.stride_tricks.sliding_window_view(mp, (4, 4), axis=(1, 2))
    m = mv[:, ::2, ::2].mean(axis=(0, 3, 4)).reshape(L)
    mm = (m == 0.0).astype(np.float32)

    # key norms via 3x3 box sum of sum_c x2p^2
    q = (x2p * x2p).sum(axis=0)               # [66, 66]
    n2 = np.zeros((H2, W2), np.float32)
    for da in range(3):
        for db in range(3):
            n2 += q[da:da + 64, db:db + 64]
    nm = (mm / np.maximum(np.sqrt(n2.reshape(L)), 1e-4)).astype(BF16)

    maf = mas.reshape(L)
    ma10 = np.ascontiguousarray(
        (SCALE * maf[:SPS]).reshape(NB, 128).T.astype(np.float32))
    maS = np.ascontiguousarray(
        (0.25 * maf[:SPS]).reshape(NB, 128).T.astype(np.float32))

    cw = np.empty((128, 36, 16), BF16)
    for g in range(GROUPS):
        for a in range(3):
            for b2 in range(3):
                cw[:, g * 9 + a * 3 + b2, :] = conv_w[g, :, :, a, b2].T.astype(BF16)

    return {
        "x1T": x1T.astype(BF16),
        "x2p": x2p.astype(BF16),
        "nmr": nm.reshape(1, L),
        "mmr": mm.astype(BF16).reshape(1, L),
        "ma10": ma10,
        "maS": maS,
        "convw": cw,
        "convb": np.ascontiguousarray(conv_b.reshape(4, 16).T).astype(np.float32),
    }


def _kernel_trn(x1, x2, mask, mask_all, conv_w, conv_b):
    from concourse.bass_utils import run_bass_kernel_spmd

    global _PROG
    if _PROG is None:
        _PROG = _build_program()

    in_maps = []
    for c in range(8):
        b, half = c // 2, c % 2
        in_maps.append(
            _prep_core(
                x1[b], x2[b], mask[b], mask_all[b, 0], conv_w, conv_b, half == 1
            )
        )
    res = run_bass_kernel_spmd(_PROG, in_maps, list(range(8)))
    out = np.empty((B, OUT_C, H1, W1), np.float32)
    for c in range(8):
        b, half = c // 2, c % 2
        o = res.results[c]["out"].reshape(OUT_C, 64, W1)
        if half:
            out[b, :, 64:] = o[:, ::-1, :]
        else:
            out[b, :, :64] = o
    return out


def kernel(x1, x2, mask, mask_all, conv_w, conv_b):
    x1 = np.asarray(x1, np.float32)
    x2 = np.asarray(x2, np.float32)
    mask = np.asarray(mask, np.float32)
    mask_all = np.asarray(mask_all, np.float32)
    conv_w = np.asarray(conv_w, np.float32)
    conv_b = np.asarray(conv_b, np.float32)
    return _kernel_trn(x1, x2, mask, mask_all, conv_w, conv_b)
